# revision 15
# baseline (speedup 1.0000x reference)
"""MoE (top-2 of 8 experts, SwiGLU) Trainium2 kernel, expert-parallel over 8 cores.

Contract: kernel(**inputs) takes the FULL unsharded inputs
  x [2,2048,1024] f32, gate_w [8,1024] f32,
  w1 [8,2048,1024] f32, w2 [8,1024,2048] f32, w3 [8,2048,1024] f32
and returns the FULL output [2,2048,1024] f32.

Strategy (expert-parallel, per the hint "replicate the gate and all-to-all the
token dispatch"): routing (gate softmax + top-2) is computed on host; tokens
are dispatched (gathered) per expert; core e runs the SwiGLU FFN of expert e
over its ~N*TOPK/E assigned tokens (padded to capacity C), pre-scaled by the
combine weight; the host scatter-adds the two expert contributions per token.

Device kernel (per core, feature-major layout so no on-device transposes;
fp32r matmuls = full PE rate at moving-dim >= 256, ~1.5e-4 matmul rel err):
  h1T = w1 @ xg^T   [H, C]   (lhsT = w1T block, rhs = xgT)
  h3T = w3 @ xg^T   [H, C]
  aT  = silu(h1T) * h3T      (ACT Silu + DVE mul, PSUM->SBUF)
  yT  = (w2 @ aT) * combine  [D, C]  (DVE mul on PSUM eviction)

Tokens are processed in free-dim chunks of 256..512 (PSUM-bank bound is 512
fp32; fp32r drops to 1/4 rate below 256), sized so the padded capacity C
hugs the max per-expert token count.
"""

import math
import sys

import numpy as np

for _p in ("/opt/trn_rl_repo", "/opt/pypackages"):
    if _p not in sys.path:
        sys.path.append(_p)

import concourse.bass as bass  # noqa: E402
import concourse.tile as tile  # noqa: E402
from concourse import bacc, mybir  # noqa: E402
from concourse.bass_utils import run_bass_kernel_spmd  # noqa: E402

B, T, D, H, E, TOPK = 2, 2048, 1024, 2048, 8, 2
N = B * T
P = 128
KD = D // P   # 8  k-tiles over D
KH = H // P   # 16 k-tiles over H
HB = H // P   # 16 h blocks of 128 (M dim, stage A)
DB = D // P   # 8  d blocks of 128 (M dim, stage B)

F32 = mybir.dt.float32
F32R = mybir.dt.float32r

# set by test.py to capture an NTFF profile; kernel() stores results here
TRACE = False
TRACE_ALL_CORES = False
LAST_RESULTS = None

_program_cache = {}

# CoreSim doesn't implement Silu; simcheck.py overrides this to Sigmoid.
_ACT_FUNC = mybir.ActivationFunctionType.Silu


def _chunk_plan(cmax: int) -> list[int]:
    """Token-chunk sizes covering cmax: each <=512 (PSUM bank), as equal as
    possible (keeps every chunk >=256 for full-rate fp32r when cmax allows),
    32-aligned, minimal total padding."""
    n = max(1, math.ceil(cmax / 512))
    chunks = []
    rem = cmax
    for i in range(n):
        s = math.ceil(rem / (n - i) / 32) * 32
        s = min(max(s, 256), 512)
        chunks.append(s)
        rem -= s
    return chunks


def _build_program(chunks: list[int]):
    """Bass program for one core: expert FFN over C = sum(chunks) tokens."""
    C = sum(chunks)
    offs = [sum(chunks[:i]) for i in range(len(chunks))]
    tsls = [bass.ds(o, s) for o, s in zip(offs, chunks)]
    nt = len(chunks)

    nc = bacc.Bacc(
        "TRN2", target_bir_lowering=False, debug=False,
        enable_asserts=False, num_devices=8,
    )
    xgT_d = nc.dram_tensor("xgT", [D, C], F32R, kind="ExternalInput").ap()
    w1T_d = nc.dram_tensor("w1T", [D, H], F32R, kind="ExternalInput").ap()
    w3T_d = nc.dram_tensor("w3T", [D, H], F32R, kind="ExternalInput").ap()
    w2T_d = nc.dram_tensor("w2T", [H, D], F32R, kind="ExternalInput").ap()
    scl_d = nc.dram_tensor("scale_b", [P, C], F32, kind="ExternalInput").ap()
    yT_d = nc.dram_tensor("yT", [D, C], F32, kind="ExternalOutput").ap()

    # DRAM views with the 128-partition k-tile split exposed
    xgT_v = xgT_d.rearrange("(k p) c -> p k c", p=P)     # [P, KD, C]
    w1T_v = w1T_d.rearrange("(k p) h -> p k h", p=P)     # [P, KD, H]
    w3T_v = w3T_d.rearrange("(k p) h -> p k h", p=P)
    w2T_v = w2T_d.rearrange("(k p) d -> p k d", p=P)     # [P, KH, D]

    with tile.TileContext(nc) as tc:
        with tc.tile_pool(name="resident", bufs=1) as res_pool, \
             tc.tile_pool(name="w13", bufs=3) as w13_pool, \
             tc.tile_pool(name="w2", bufs=3) as w2_pool, \
             tc.tile_pool(name="ev", bufs=3) as ev_pool, \
             tc.tile_pool(name="psum", bufs=2, space="PSUM") as ps_pool:

            # xg loaded in (token-chunk, k) slices so stage A's first psum
            # group can finish after ~1/nt of the 4.5 MB load instead of
            # waiting for all of it (subtile deps make the MMs wait only on
            # their own slice).
            # Head loads are HBM-bandwidth-bound: only xg token-chunk 0 and
            # the first weight group gate the first matmuls; the rest of xg
            # streams while chunk-0 compute runs. DMAs are split over two
            # queues (w1 on sync; xg+w3 on gpsimd), each tile written by a
            # single queue (mixing queues within one tile loses the
            # DMA->matmul ordering).
            xg = res_pool.tile([P, KD, C], F32R, tag="xg")
            for k in range(KD):
                nc.gpsimd.dma_start(xg[:, k, tsls[0]], xgT_v[:, k, tsls[0]])
            act = res_pool.tile([P, KH, C], F32R, tag="act")

            # ---- stage A: act[H, C] = silu(w1 @ xgT) * (w3 @ xgT) ----
            # h-blocks in resident groups of HQ (weights stay in SBUF for all
            # token chunks) with the token-chunk loop outer, so chunk t+1 only
            # becomes critical after HQ h-blocks of chunk-t compute.
            HQ = 4
            for q in range(0, HB, HQ):
                hs = list(range(q, min(q + HQ, HB)))
                w1ts, w3ts = [], []
                for i, h in enumerate(hs):
                    w1t = w13_pool.tile([P, KD, P], F32R, tag=f"w1_{i}",
                                        bufs=2, name=f"w1t_{h}")
                    nc.sync.dma_start(w1t[:], w1T_v[:, :, h * P:(h + 1) * P])
                    w3t = w13_pool.tile([P, KD, P], F32R, tag=f"w3_{i}",
                                        bufs=2, name=f"w3t_{h}")
                    nc.scalar.dma_start(w3t[:], w3T_v[:, :, h * P:(h + 1) * P])
                    w1ts.append(w1t)
                    w3ts.append(w3t)
                if q == 0:  # stream the remaining xg chunks behind group 0
                    for tt in range(1, nt):
                        for k in range(KD):
                            nc.gpsimd.dma_start(xg[:, k, tsls[tt]],
                                                xgT_v[:, k, tsls[tt]])
                for t in range(nt):
                    tsl = tsls[t]
                    for i, h in enumerate(hs):
                        ph1 = ps_pool.tile([P, chunks[t]], F32, tag="h1",
                                           bufs=3, name=f"ph1_{h}_{t}")
                        for k in range(KD):
                            nc.tensor.matmul(ph1[:], w1ts[i][:, k, :],
                                             xg[:, k, tsl],
                                             start=(k == 0), stop=(k == KD - 1))
                        ph3 = ps_pool.tile([P, chunks[t]], F32, tag="h3",
                                           bufs=3, name=f"ph3_{h}_{t}")
                        for k in range(KD):
                            nc.tensor.matmul(ph3[:], w3ts[i][:, k, :],
                                             xg[:, k, tsl],
                                             start=(k == 0), stop=(k == KD - 1))
                        asl = act[:, h, tsl]
                        nc.scalar.activation(asl, ph1[:], func=_ACT_FUNC)
                        nc.vector.tensor_mul(asl, asl, ph3[:])

            # combine-weight row (needed only for stage B evictions)
            scl = res_pool.tile([P, C], F32, tag="scl")
            nc.gpsimd.dma_start(scl[:], scl_d[:, :])

            # ---- stage B: yT[D, C] = (w2 @ act) * scale ----
            for d in range(DB):
                w2t = w2_pool.tile([P, KH, P], F32R, tag="w2")
                nc.sync.dma_start(w2t[:], w2T_v[:, :, d * P:(d + 1) * P])
                for t in range(nt):
                    tsl = tsls[t]
                    py = ps_pool.tile([P, chunks[t]], F32, tag="y")
                    for k in range(KH):
                        nc.tensor.matmul(py[:], w2t[:, k, :], act[:, k, tsl],
                                         start=(k == 0), stop=(k == KH - 1))
                    ysb = ev_pool.tile([P, chunks[t]], F32, tag="ysb")
                    nc.vector.tensor_mul(ysb[:], py[:], scl[:, tsl])
                    nc.scalar.dma_start(yT_d[d * P:(d + 1) * P, tsl], ysb[:])

    nc.compile()
    return nc


def _route(flat, gate_w):
    """Host replica of the reference router. Returns top-2 expert ids and
    combine weights (top-2 of softmax, renormalized)."""
    logits = flat @ gate_w.T                                   # [N, E] f32
    m = logits.max(axis=1, keepdims=True)
    p = np.exp((logits - m).astype(np.float32))
    probs = p / p.sum(axis=1, keepdims=True)
    idx = np.argsort(-probs, axis=1, kind="stable")[:, :TOPK]  # [N, 2]
    top = np.take_along_axis(probs, idx, axis=1)               # [N, 2]
    wn = top / top.sum(axis=1, keepdims=True)
    return idx, wn


def kernel(x, gate_w, w1, w2, w3):
    global LAST_RESULTS
    x = np.asarray(x, np.float32)
    gate_w = np.asarray(gate_w, np.float32)
    w1 = np.asarray(w1, np.float32)
    w2 = np.asarray(w2, np.float32)
    w3 = np.asarray(w3, np.float32)

    flat = x.reshape(N, D)
    idx, wn = _route(flat, gate_w)

    sels, wsels = [], []
    for e in range(E):
        hit = idx == e                                         # [N, 2]
        sel = np.nonzero(hit.any(axis=1))[0]
        k = hit[sel, 1].astype(np.int64)                       # which top slot
        sels.append(sel)
        wsels.append(wn[sel, k])
    cmax = max(len(s) for s in sels)
    chunks = _chunk_plan(cmax)
    C = sum(chunks)

    xT = np.ascontiguousarray(flat.T)                          # [D, N]
    in_maps = []
    for e in range(E):
        sel = sels[e]
        xgT = np.zeros((D, C), np.float32)
        xgT[:, :len(sel)] = xT[:, sel]
        scale_b = np.zeros((P, C), np.float32)
        scale_b[:, :len(sel)] = wsels[e][None, :]
        in_maps.append({
            "xgT": xgT,
            "w1T": np.ascontiguousarray(w1[e].T),
            "w3T": np.ascontiguousarray(w3[e].T),
            "w2T": np.ascontiguousarray(w2[e].T),
            "scale_b": scale_b,
        })

    key = tuple(chunks)
    if key not in _program_cache:
        _program_cache[key] = _build_program(chunks)
    nc = _program_cache[key]

    res = run_bass_kernel_spmd(
        nc, in_maps, core_ids=list(range(E)),
        trace=TRACE,
        trace_cores=list(range(E)) if (TRACE and TRACE_ALL_CORES) else None,
    )
    LAST_RESULTS = res

    out = np.zeros((N, D), np.float32)
    for e in range(E):
        sel = sels[e]
        out[sel] += res.results[e]["yT"][:, :len(sel)].T
    return out.reshape(B, T, D)


# revision 16
# speedup vs baseline: 1.0104x; 1.0104x over previous
"""MoE (top-2 of 8 experts, SwiGLU) Trainium2 kernel, expert-parallel over 8 cores.

Contract: kernel(**inputs) takes the FULL unsharded inputs
  x [2,2048,1024] f32, gate_w [8,1024] f32,
  w1 [8,2048,1024] f32, w2 [8,1024,2048] f32, w3 [8,2048,1024] f32
and returns the FULL output [2,2048,1024] f32.

Strategy (expert-parallel, per the hint "replicate the gate and all-to-all the
token dispatch"): routing (gate softmax + top-2) is computed on host; tokens
are dispatched (gathered) per expert; core e runs the SwiGLU FFN of expert e
over its ~N*TOPK/E assigned tokens (padded to capacity C), pre-scaled by the
combine weight; the host scatter-adds the two expert contributions per token.

Device kernel (per core, feature-major layout so no on-device transposes;
fp32r matmuls = full PE rate at moving-dim >= 256, ~1.5e-4 matmul rel err):
  h1T = w1 @ xg^T   [H, C]   (lhsT = w1T block, rhs = xgT)
  h3T = w3 @ xg^T   [H, C]
  aT  = silu(h1T) * h3T      (ACT Silu + DVE mul, PSUM->SBUF)
  yT  = (w2 @ aT) * combine  [D, C]  (DVE mul on PSUM eviction)

Tokens are processed in free-dim chunks of 256..512 (PSUM-bank bound is 512
fp32; fp32r drops to 1/4 rate below 256), sized so the padded capacity C
hugs the max per-expert token count.
"""

import math
import sys

import numpy as np

for _p in ("/opt/trn_rl_repo", "/opt/pypackages"):
    if _p not in sys.path:
        sys.path.append(_p)

import concourse.bass as bass  # noqa: E402
import concourse.tile as tile  # noqa: E402
from concourse import bacc, mybir  # noqa: E402
from concourse.bass_utils import run_bass_kernel_spmd  # noqa: E402

B, T, D, H, E, TOPK = 2, 2048, 1024, 2048, 8, 2
N = B * T
P = 128
KD = D // P   # 8  k-tiles over D
KH = H // P   # 16 k-tiles over H
HB = H // P   # 16 h blocks of 128 (M dim, stage A)
DB = D // P   # 8  d blocks of 128 (M dim, stage B)

F32 = mybir.dt.float32
F32R = mybir.dt.float32r

# set by test.py to capture an NTFF profile; kernel() stores results here
TRACE = False
TRACE_ALL_CORES = False
LAST_RESULTS = None

_program_cache = {}

# CoreSim doesn't implement Silu; simcheck.py overrides this to Sigmoid.
_ACT_FUNC = mybir.ActivationFunctionType.Silu


def _chunk_plan(cmax: int) -> list[int]:
    """Token-chunk sizes covering cmax: each <=512 (PSUM bank), as equal as
    possible (keeps every chunk >=256 for full-rate fp32r when cmax allows),
    32-aligned, minimal total padding."""
    n = max(1, math.ceil(cmax / 512))
    chunks = []
    rem = cmax
    for i in range(n):
        s = math.ceil(rem / (n - i) / 32) * 32
        s = min(max(s, 256), 512)
        chunks.append(s)
        rem -= s
    return chunks


def _build_program(chunks: list[int]):
    """Bass program for one core: expert FFN over C = sum(chunks) tokens."""
    C = sum(chunks)
    offs = [sum(chunks[:i]) for i in range(len(chunks))]
    tsls = [bass.ds(o, s) for o, s in zip(offs, chunks)]
    nt = len(chunks)

    nc = bacc.Bacc(
        "TRN2", target_bir_lowering=False, debug=False,
        enable_asserts=False, num_devices=8,
    )
    xgT_d = nc.dram_tensor("xgT", [D, C], F32R, kind="ExternalInput").ap()
    w1T_d = nc.dram_tensor("w1T", [D, H], F32R, kind="ExternalInput").ap()
    w3T_d = nc.dram_tensor("w3T", [D, H], F32R, kind="ExternalInput").ap()
    w2T_d = nc.dram_tensor("w2T", [H, D], F32R, kind="ExternalInput").ap()
    scl_d = nc.dram_tensor("scale_b", [P, C], F32, kind="ExternalInput").ap()
    yT_d = nc.dram_tensor("yT", [D, C], F32, kind="ExternalOutput").ap()

    # DRAM views with the 128-partition k-tile split exposed
    xgT_v = xgT_d.rearrange("(k p) c -> p k c", p=P)     # [P, KD, C]
    w1T_v = w1T_d.rearrange("(k p) h -> p k h", p=P)     # [P, KD, H]
    w3T_v = w3T_d.rearrange("(k p) h -> p k h", p=P)
    w2T_v = w2T_d.rearrange("(k p) d -> p k d", p=P)     # [P, KH, D]

    with tile.TileContext(nc) as tc:
        with tc.tile_pool(name="resident", bufs=1) as res_pool, \
             tc.tile_pool(name="w13", bufs=3) as w13_pool, \
             tc.tile_pool(name="w2", bufs=3) as w2_pool, \
             tc.tile_pool(name="ev", bufs=3) as ev_pool, \
             tc.tile_pool(name="psum", bufs=2, space="PSUM") as ps_pool:

            # xg loaded in (token-chunk, k) slices so stage A's first psum
            # group can finish after ~1/nt of the 4.5 MB load instead of
            # waiting for all of it (subtile deps make the MMs wait only on
            # their own slice).
            # Head loads are HBM-bandwidth-bound: only xg token-chunk 0 and
            # the first weight group gate the first matmuls; the rest of xg
            # streams while chunk-0 compute runs. DMAs are split over two
            # queues (w1 on sync; xg+w3 on gpsimd), each tile written by a
            # single queue (mixing queues within one tile loses the
            # DMA->matmul ordering).
            xg = res_pool.tile([P, KD, C], F32R, tag="xg")
            for k in range(KD):
                nc.gpsimd.dma_start(xg[:, k, tsls[0]], xgT_v[:, k, tsls[0]])
            act = res_pool.tile([P, KH, C], F32R, tag="act")

            # ---- stage A: act[H, C] = silu(w1 @ xgT) * (w3 @ xgT) ----
            # h-blocks in resident groups of HQ (weights stay in SBUF for all
            # token chunks) with the token-chunk loop outer, so chunk t+1 only
            # becomes critical after HQ h-blocks of chunk-t compute.
            HQ = 4
            for q in range(0, HB, HQ):
                hs = list(range(q, min(q + HQ, HB)))
                w1ts, w3ts = [], []
                for i, h in enumerate(hs):
                    w1t = w13_pool.tile([P, KD, P], F32R, tag=f"w1_{i}",
                                        bufs=2, name=f"w1t_{h}")
                    nc.sync.dma_start(w1t[:], w1T_v[:, :, h * P:(h + 1) * P])
                    w3t = w13_pool.tile([P, KD, P], F32R, tag=f"w3_{i}",
                                        bufs=2, name=f"w3t_{h}")
                    nc.sync.dma_start(w3t[:], w3T_v[:, :, h * P:(h + 1) * P])
                    w1ts.append(w1t)
                    w3ts.append(w3t)
                if q == 0:  # stream the remaining xg chunks behind group 0
                    for tt in range(1, nt):
                        for k in range(KD):
                            nc.gpsimd.dma_start(xg[:, k, tsls[tt]],
                                                xgT_v[:, k, tsls[tt]])
                for t in range(nt):
                    tsl = tsls[t]
                    for i, h in enumerate(hs):
                        ph1 = ps_pool.tile([P, chunks[t]], F32, tag="h1",
                                           bufs=3, name=f"ph1_{h}_{t}")
                        for k in range(KD):
                            nc.tensor.matmul(ph1[:], w1ts[i][:, k, :],
                                             xg[:, k, tsl],
                                             start=(k == 0), stop=(k == KD - 1))
                        ph3 = ps_pool.tile([P, chunks[t]], F32, tag="h3",
                                           bufs=3, name=f"ph3_{h}_{t}")
                        for k in range(KD):
                            nc.tensor.matmul(ph3[:], w3ts[i][:, k, :],
                                             xg[:, k, tsl],
                                             start=(k == 0), stop=(k == KD - 1))
                        asl = act[:, h, tsl]
                        nc.scalar.activation(asl, ph1[:], func=_ACT_FUNC)
                        nc.vector.tensor_mul(asl, asl, ph3[:])

            # combine-weight row (needed only for stage B evictions)
            scl = res_pool.tile([P, C], F32, tag="scl")
            nc.gpsimd.dma_start(scl[:], scl_d[:, :])

            # ---- stage B: yT[D, C] = (w2 @ act) * scale ----
            for d in range(DB):
                w2t = w2_pool.tile([P, KH, P], F32R, tag="w2")
                nc.sync.dma_start(w2t[:], w2T_v[:, :, d * P:(d + 1) * P])
                for t in range(nt):
                    tsl = tsls[t]
                    py = ps_pool.tile([P, chunks[t]], F32, tag="y")
                    for k in range(KH):
                        nc.tensor.matmul(py[:], w2t[:, k, :], act[:, k, tsl],
                                         start=(k == 0), stop=(k == KH - 1))
                    ysb = ev_pool.tile([P, chunks[t]], F32, tag="ysb")
                    nc.vector.tensor_mul(ysb[:], py[:], scl[:, tsl])
                    nc.scalar.dma_start(yT_d[d * P:(d + 1) * P, tsl], ysb[:])

    nc.compile()
    return nc


def _route(flat, gate_w):
    """Host replica of the reference router. Returns top-2 expert ids and
    combine weights (top-2 of softmax, renormalized)."""
    logits = flat @ gate_w.T                                   # [N, E] f32
    m = logits.max(axis=1, keepdims=True)
    p = np.exp((logits - m).astype(np.float32))
    probs = p / p.sum(axis=1, keepdims=True)
    idx = np.argsort(-probs, axis=1, kind="stable")[:, :TOPK]  # [N, 2]
    top = np.take_along_axis(probs, idx, axis=1)               # [N, 2]
    wn = top / top.sum(axis=1, keepdims=True)
    return idx, wn


def kernel(x, gate_w, w1, w2, w3):
    global LAST_RESULTS
    x = np.asarray(x, np.float32)
    gate_w = np.asarray(gate_w, np.float32)
    w1 = np.asarray(w1, np.float32)
    w2 = np.asarray(w2, np.float32)
    w3 = np.asarray(w3, np.float32)

    flat = x.reshape(N, D)
    idx, wn = _route(flat, gate_w)

    sels, wsels = [], []
    for e in range(E):
        hit = idx == e                                         # [N, 2]
        sel = np.nonzero(hit.any(axis=1))[0]
        k = hit[sel, 1].astype(np.int64)                       # which top slot
        sels.append(sel)
        wsels.append(wn[sel, k])
    cmax = max(len(s) for s in sels)
    chunks = _chunk_plan(cmax)
    C = sum(chunks)

    xT = np.ascontiguousarray(flat.T)                          # [D, N]
    in_maps = []
    for e in range(E):
        sel = sels[e]
        xgT = np.zeros((D, C), np.float32)
        xgT[:, :len(sel)] = xT[:, sel]
        scale_b = np.zeros((P, C), np.float32)
        scale_b[:, :len(sel)] = wsels[e][None, :]
        in_maps.append({
            "xgT": xgT,
            "w1T": np.ascontiguousarray(w1[e].T),
            "w3T": np.ascontiguousarray(w3[e].T),
            "w2T": np.ascontiguousarray(w2[e].T),
            "scale_b": scale_b,
        })

    key = tuple(chunks)
    if key not in _program_cache:
        _program_cache[key] = _build_program(chunks)
    nc = _program_cache[key]

    res = run_bass_kernel_spmd(
        nc, in_maps, core_ids=list(range(E)),
        trace=TRACE,
        trace_cores=list(range(E)) if (TRACE and TRACE_ALL_CORES) else None,
    )
    LAST_RESULTS = res

    out = np.zeros((N, D), np.float32)
    for e in range(E):
        sel = sels[e]
        out[sel] += res.results[e]["yT"][:, :len(sel)].T
    return out.reshape(B, T, D)


# revision 17
# speedup vs baseline: 1.0386x; 1.0279x over previous
"""MoE (top-2 of 8 experts, SwiGLU) Trainium2 kernel, expert-parallel over 8 cores.

Contract: kernel(**inputs) takes the FULL unsharded inputs
  x [2,2048,1024] f32, gate_w [8,1024] f32,
  w1 [8,2048,1024] f32, w2 [8,1024,2048] f32, w3 [8,2048,1024] f32
and returns the FULL output [2,2048,1024] f32.

Strategy (expert-parallel, per the hint "replicate the gate and all-to-all the
token dispatch"): routing (gate softmax + top-2) is computed on host; tokens
are dispatched (gathered) per expert; core e runs the SwiGLU FFN of expert e
over its ~N*TOPK/E assigned tokens (padded to capacity C), pre-scaled by the
combine weight; the host scatter-adds the two expert contributions per token.

Device kernel (per core, feature-major layout so no on-device transposes;
fp32r matmuls = full PE rate at moving-dim >= 256, ~1.5e-4 matmul rel err):
  h1T = w1 @ xg^T   [H, C]   (lhsT = w1T block, rhs = xgT)
  h3T = w3 @ xg^T   [H, C]
  aT  = silu(h1T) * h3T      (ACT Silu + DVE mul, PSUM->SBUF)
  yT  = (w2 @ aT) * combine  [D, C]  (DVE mul on PSUM eviction)

Tokens are processed in free-dim chunks of 256..512 (PSUM-bank bound is 512
fp32; fp32r drops to 1/4 rate below 256), sized so the padded capacity C
hugs the max per-expert token count.
"""

import math
import sys

import numpy as np

for _p in ("/opt/trn_rl_repo", "/opt/pypackages"):
    if _p not in sys.path:
        sys.path.append(_p)

import concourse.bass as bass  # noqa: E402
import concourse.tile as tile  # noqa: E402
from concourse import bacc, mybir  # noqa: E402
from concourse.bass_utils import run_bass_kernel_spmd  # noqa: E402

B, T, D, H, E, TOPK = 2, 2048, 1024, 2048, 8, 2
N = B * T
P = 128
KD = D // P   # 8  k-tiles over D
KH = H // P   # 16 k-tiles over H
HB = H // P   # 16 h blocks of 128 (M dim, stage A)
DB = D // P   # 8  d blocks of 128 (M dim, stage B)

F32 = mybir.dt.float32
F32R = mybir.dt.float32r

# set by test.py to capture an NTFF profile; kernel() stores results here
TRACE = False
TRACE_ALL_CORES = False
LAST_RESULTS = None

_program_cache = {}

# CoreSim doesn't implement Silu; simcheck.py overrides this to Sigmoid.
_ACT_FUNC = mybir.ActivationFunctionType.Silu


def _chunk_plan(cmax: int) -> list[int]:
    """Token-chunk sizes covering cmax: each <=512 (PSUM bank), as equal as
    possible (keeps every chunk >=256 for full-rate fp32r when cmax allows),
    32-aligned, minimal total padding."""
    n = max(1, math.ceil(cmax / 512))
    chunks = []
    rem = cmax
    for i in range(n):
        s = math.ceil(rem / (n - i) / 32) * 32
        s = min(max(s, 256), 512)
        chunks.append(s)
        rem -= s
    return chunks


def _build_program(chunks: list[int]):
    """Bass program for one core: expert FFN over C = sum(chunks) tokens."""
    C = sum(chunks)
    offs = [sum(chunks[:i]) for i in range(len(chunks))]
    tsls = [bass.ds(o, s) for o, s in zip(offs, chunks)]
    nt = len(chunks)

    nc = bacc.Bacc(
        "TRN2", target_bir_lowering=False, debug=False,
        enable_asserts=False, num_devices=8,
    )
    xgT_d = nc.dram_tensor("xgT", [D, C], F32R, kind="ExternalInput").ap()
    w1T_d = nc.dram_tensor("w1T", [D, H], F32R, kind="ExternalInput").ap()
    w3T_d = nc.dram_tensor("w3T", [D, H], F32R, kind="ExternalInput").ap()
    w2T_d = nc.dram_tensor("w2T", [H, D], F32R, kind="ExternalInput").ap()
    scl_d = nc.dram_tensor("scale_b", [P, C], F32, kind="ExternalInput").ap()
    yT_d = nc.dram_tensor("yT", [D, C], F32, kind="ExternalOutput").ap()

    # DRAM views with the 128-partition k-tile split exposed
    xgT_v = xgT_d.rearrange("(k p) c -> p k c", p=P)     # [P, KD, C]
    w1T_v = w1T_d.rearrange("(k p) h -> p k h", p=P)     # [P, KD, H]
    w3T_v = w3T_d.rearrange("(k p) h -> p k h", p=P)
    w2T_v = w2T_d.rearrange("(k p) d -> p k d", p=P)     # [P, KH, D]

    with tile.TileContext(nc) as tc:
        with tc.tile_pool(name="resident", bufs=1) as res_pool, \
             tc.tile_pool(name="w13", bufs=3) as w13_pool, \
             tc.tile_pool(name="w2", bufs=3) as w2_pool, \
             tc.tile_pool(name="ev", bufs=3) as ev_pool, \
             tc.tile_pool(name="psum", bufs=2, space="PSUM") as ps_pool:

            # xg loaded in (token-chunk, k) slices so stage A's first psum
            # group can finish after ~1/nt of the 4.5 MB load instead of
            # waiting for all of it (subtile deps make the MMs wait only on
            # their own slice).
            # xg loaded in (token-chunk, k) slices on the gpsimd DMA queue so
            # stage A's first psum groups only gate on their own slice, while
            # the weight stream runs in parallel on the sync queue.
            xg = res_pool.tile([P, KD, C], F32R, tag="xg")
            for t in range(nt):
                for k in range(KD):
                    nc.gpsimd.dma_start(xg[:, k, tsls[t]], xgT_v[:, k, tsls[t]])
            act = res_pool.tile([P, KH, C], F32R, tag="act")

            # ---- stage A: act[H, C] = silu(w1 @ xgT) * (w3 @ xgT) ----
            for h in range(HB):
                w1t = w13_pool.tile([P, KD, P], F32R, tag="w1")
                nc.sync.dma_start(w1t[:], w1T_v[:, :, h * P:(h + 1) * P])
                w3t = w13_pool.tile([P, KD, P], F32R, tag="w3")
                nc.sync.dma_start(w3t[:], w3T_v[:, :, h * P:(h + 1) * P])
                for t in range(nt):
                    tsl = tsls[t]
                    ph1 = ps_pool.tile([P, chunks[t]], F32, tag="h1", bufs=3)
                    for k in range(KD):
                        nc.tensor.matmul(ph1[:], w1t[:, k, :], xg[:, k, tsl],
                                         start=(k == 0), stop=(k == KD - 1))
                    ph3 = ps_pool.tile([P, chunks[t]], F32, tag="h3", bufs=3)
                    for k in range(KD):
                        nc.tensor.matmul(ph3[:], w3t[:, k, :], xg[:, k, tsl],
                                         start=(k == 0), stop=(k == KD - 1))
                    asl = act[:, h, tsl]
                    nc.scalar.activation(asl, ph1[:], func=_ACT_FUNC)
                    nc.vector.tensor_mul(asl, asl, ph3[:])

            # combine-weight row (needed only for stage B evictions)
            scl = res_pool.tile([P, C], F32, tag="scl")
            nc.gpsimd.dma_start(scl[:], scl_d[:, :])

            # ---- stage B: yT[D, C] = (w2 @ act) * scale ----
            for d in range(DB):
                w2t = w2_pool.tile([P, KH, P], F32R, tag="w2")
                nc.sync.dma_start(w2t[:], w2T_v[:, :, d * P:(d + 1) * P])
                for t in range(nt):
                    tsl = tsls[t]
                    py = ps_pool.tile([P, chunks[t]], F32, tag="y")
                    for k in range(KH):
                        nc.tensor.matmul(py[:], w2t[:, k, :], act[:, k, tsl],
                                         start=(k == 0), stop=(k == KH - 1))
                    ysb = ev_pool.tile([P, chunks[t]], F32, tag="ysb")
                    nc.vector.tensor_mul(ysb[:], py[:], scl[:, tsl])
                    nc.scalar.dma_start(yT_d[d * P:(d + 1) * P, tsl], ysb[:])

    nc.compile()
    return nc


def _route(flat, gate_w):
    """Host replica of the reference router. Returns top-2 expert ids and
    combine weights (top-2 of softmax, renormalized)."""
    logits = flat @ gate_w.T                                   # [N, E] f32
    m = logits.max(axis=1, keepdims=True)
    p = np.exp((logits - m).astype(np.float32))
    probs = p / p.sum(axis=1, keepdims=True)
    idx = np.argsort(-probs, axis=1, kind="stable")[:, :TOPK]  # [N, 2]
    top = np.take_along_axis(probs, idx, axis=1)               # [N, 2]
    wn = top / top.sum(axis=1, keepdims=True)
    return idx, wn


def kernel(x, gate_w, w1, w2, w3):
    global LAST_RESULTS
    x = np.asarray(x, np.float32)
    gate_w = np.asarray(gate_w, np.float32)
    w1 = np.asarray(w1, np.float32)
    w2 = np.asarray(w2, np.float32)
    w3 = np.asarray(w3, np.float32)

    flat = x.reshape(N, D)
    idx, wn = _route(flat, gate_w)

    sels, wsels = [], []
    for e in range(E):
        hit = idx == e                                         # [N, 2]
        sel = np.nonzero(hit.any(axis=1))[0]
        k = hit[sel, 1].astype(np.int64)                       # which top slot
        sels.append(sel)
        wsels.append(wn[sel, k])
    cmax = max(len(s) for s in sels)
    chunks = _chunk_plan(cmax)
    C = sum(chunks)

    xT = np.ascontiguousarray(flat.T)                          # [D, N]
    in_maps = []
    for e in range(E):
        sel = sels[e]
        xgT = np.zeros((D, C), np.float32)
        xgT[:, :len(sel)] = xT[:, sel]
        scale_b = np.zeros((P, C), np.float32)
        scale_b[:, :len(sel)] = wsels[e][None, :]
        in_maps.append({
            "xgT": xgT,
            "w1T": np.ascontiguousarray(w1[e].T),
            "w3T": np.ascontiguousarray(w3[e].T),
            "w2T": np.ascontiguousarray(w2[e].T),
            "scale_b": scale_b,
        })

    key = tuple(chunks)
    if key not in _program_cache:
        _program_cache[key] = _build_program(chunks)
    nc = _program_cache[key]

    res = run_bass_kernel_spmd(
        nc, in_maps, core_ids=list(range(E)),
        trace=TRACE,
        trace_cores=list(range(E)) if (TRACE and TRACE_ALL_CORES) else None,
    )
    LAST_RESULTS = res

    out = np.zeros((N, D), np.float32)
    for e in range(E):
        sel = sels[e]
        out[sel] += res.results[e]["yT"][:, :len(sel)].T
    return out.reshape(B, T, D)
